# revision 19
# baseline (speedup 1.0000x reference)
"""Trainium2 Bass kernel for an 8-layer densely-connected MLP (v2).

Math: the reference's dense past/future skip-connection structure is linear
in the per-layer silu outputs a_i, so it collapses (host-side, fp64) to

    a_0 = silu(x @ W0.T + b0)
    a_i = silu(sum_{m<i} a_m @ P[i][m].T + bh[i-1])      i = 1..7
    out = log_softmax(a_7 @ Wout.T + bout)

with 28 precomputed 64x64 matrices P[i][m].

Device layout: activations are feature-major tiles T[i] of [128, chunk]
where each megatile of `mega` batch rows is two chunks A/B of mega/2 rows
living in partition halves.  The half assignment ALTERNATES with layer
parity (even i: A in partitions 0:64; odd i: B in 0:64).  With stationaries
duplicated into both partition halves, every dense term (i, m) maps to two
independent 64x64 PE quadrant matmuls (row group = m%2 side, column group =
i%2 side), so consecutive m terms tile all four PE quadrants with no
activation copies at all.

The schedule software-pipelines ~4 megatiles (round-robin, one layer per
visit, admission paced to the x DMA stream) so the PE engine never drains:
TRN2's PE p-state ramps 0.65->1.2->2.4 GHz only under continuous execution,
so avoiding stalls literally doubles the clock.  Logits are transposed on
the PE in fp16 (fp32 transposes run the array at half rate), PSUM->SBUF
logit staging rides the otherwise-idle gpsimd engine, and log-softmax is
deferred to one batch (a single Exp/Ln table swap) plus a tiny tail segment.
"""

import sys

sys.path.insert(0, "/opt/trn_rl_repo")

import numpy as np

from contextlib import ExitStack

from concourse import bass, mybir, tile
from concourse.bass_utils import run_bass_kernel_spmd

# Problem constants (hardcoded per harness contract)
B, IN, H, OUT, L = 65536, 784, 64, 10, 8
N_CORES = 8
B_CORE = B // N_CORES            # 8192
KBLK = 7                         # K blocks for layer 0
KP = 128                         # padded K-block height (784 -> 7*128)

# Megatile sizes: small at the ends (fast pipeline fill, short tail).
MEGA_SCHED = [256, 512, 1024, 1024, 1024, 1024, 1024, 1024, 512, 512, 256]
assert sum(MEGA_SCHED) == B_CORE

f16 = mybir.dt.float16
f32 = mybir.dt.float32
AF = mybir.ActivationFunctionType

W_INFLIGHT = 4                   # megatiles concurrently in the dense pipe
NGEN = W_INFLIGHT + 1            # T-tile buffer generations

# emission-time estimates used only to pace megatile admission (build-time)
EST_T0_NS = 6500.0               # preamble before first matmul retires
EST_ARR0_NS = 5500.0             # DMA latency before first x byte lands
EST_BW = 0.345                   # bytes/ns sustained x DMA bandwidth
EST_SLOT_OVH = 55.0              # per-slot fixed overhead (ns)
EST_CYC = 0.42                   # ns per moving column at ramped clock


def make_sched(sizes):
    meta, start, aoff = [], 0, 0
    for mg, mega in enumerate(sizes):
        nblk = mega // 128
        meta.append(
            dict(mg=mg, mega=mega, start=start, chunk=mega // 2, nblk=nblk, aoff=aoff)
        )
        start += mega
        aoff += nblk * OUT
    return meta


SCHED = make_sched(MEGA_SCHED)
ACOLS = sum(m["nblk"] * OUT for m in SCHED)   # 640
NBLK_MAX = max(m["nblk"] for m in SCHED)      # 8
NPAIR = L * (L - 1) // 2                      # 28 (i, m) blocks
PIDX = {}
for _i in range(1, L):
    for _m in range(_i):
        PIDX[(_i, _m)] = len(PIDX)


def _xoff(sched):
    """Column offset of each megatile slab in the per-core x tensor.
    Slab layout per partition: [ck, j, c] contiguous."""
    offs, off = {}, 0
    for m in sched:
        offs[m["mg"]] = off
        off += 2 * KBLK * m["chunk"]
    return offs, off


XOFFS, XCOLS = _xoff(SCHED)


# ----------------------------------------------------------------------------
# Host-side weight preprocessing
# ----------------------------------------------------------------------------

def _precompute_P(Wh, bh, Wp, Wf):
    """Collapse past/future dense structure into P[(i, m)] (fp64)."""
    Wh = Wh.astype(np.float64)
    Wp = Wp.astype(np.float64)
    Wf = Wf.astype(np.float64)
    nl = L
    Z = np.zeros((H, H))
    S = {}
    for k in range(nl):
        for i in range(nl):
            S[(k, i)] = sum((Wf[k * (nl - 1) + (j - 1)] for j in range(i + 1, nl)), start=Z)
    G = {(0, 0): np.eye(H)}
    for i in range(1, nl):
        G[(i, i)] = np.eye(H) + S[(i, i)] if i < nl - 1 else np.eye(H)
        for m in range(i):
            G[(i, m)] = sum((S[(k, i)] @ G[(k, m)] for k in range(m, i)), start=Z)
    P = {}
    for i in range(1, nl):
        C = {j: Wh[i - 1] @ Wp[j * (nl - 1) + (i - 1)] for j in range(i)}
        for m in range(i):
            P[(i, m)] = sum((C[j] @ G[(j, m)] for j in range(m, i)), start=Z)
    return P


def _pack_weights(W0, b0, Wh, bh, Wp, Wf, Wout, bout):
    P = _precompute_P(Wh, bh, Wp, Wf)
    # W0.T in K-blocks padded 112 -> 128 rows: [128, 7, 64]
    w0t = np.zeros((KP, KBLK, H), np.float16)
    w0t[:112] = np.ascontiguousarray(
        W0.astype(np.float64).T.reshape(KBLK, 112, H).transpose(1, 0, 2)
    ).astype(np.float16)
    # Dense stationaries: full 128-row stationaries per (i, m, stream) with
    # the half that multiplies the other stream's features zeroed, so every
    # matmul is the proven full-row column-group pattern (no PE row tiling).
    # Stream s=0 consumes the A-half of T[m] (rows 64*(m%2)); s=1 the B-half.
    wd = np.zeros((128, 2 * NPAIR, H), np.float16)
    for (i, m), p in PIDX.items():
        pt = P[(i, m)].T.astype(np.float16)
        ra = 64 * (m % 2)
        wd[ra : ra + H, 2 * p] = pt
        wd[64 - ra : 128 - ra, 2 * p + 1] = pt
    woutt_d = np.zeros((128, OUT), np.float16)
    woutt_d[0:H] = Wout.T.astype(np.float16)
    woutt_d[H:128] = Wout.T.astype(np.float16)
    # per-layer biases duplicated into both halves: [128, 8]
    bias8 = np.zeros((128, L), np.float32)
    bias8[0:H, 0] = b0
    bias8[H:128, 0] = b0
    for i in range(1, L):
        bias8[0:H, i] = bh[i - 1]
        bias8[H:128, i] = bh[i - 1]
    boutb = np.tile(bout.astype(np.float32), (128, NBLK_MAX))
    ident = np.eye(OUT, dtype=np.float16)
    ident32 = np.eye(OUT, dtype=np.float32)
    return dict(
        w0t=w0t, wd=wd, woutt_d=woutt_d, bias8=bias8, boutb=boutb, ident=ident,
        ident32=ident32,
    )


# ----------------------------------------------------------------------------
# Device program
# ----------------------------------------------------------------------------

def build_nc(sched=None, silu_via_sigmoid=False, fp16_tp=True, upfront_dma=True):
    nc = bass.Bass()
    ftp = f16 if fp16_tp else f32
    sched = SCHED if sched is None else sched
    nmt = len(sched)
    acols = sum(m["nblk"] * OUT for m in sched)
    xoffs, xcols = _xoff(sched)

    xt_e = nc.dram_tensor("xt", [KP, xcols], f16, kind="ExternalInput")
    w0t_e = nc.dram_tensor("w0t", [KP, KBLK, H], f16, kind="ExternalInput")
    wd_e = nc.dram_tensor("wd", [128, 2 * NPAIR, H], f16, kind="ExternalInput")
    woutt_e = nc.dram_tensor("woutt_d", [128, OUT], f16, kind="ExternalInput")
    bias8_e = nc.dram_tensor("bias8", [128, L], f32, kind="ExternalInput")
    boutb_e = nc.dram_tensor("boutb", [128, OUT * NBLK_MAX], f32, kind="ExternalInput")
    ident_e = nc.dram_tensor("ident", [OUT, OUT], f16, kind="ExternalInput")
    ident32_e = nc.dram_tensor("ident32", [OUT, OUT], f32, kind="ExternalInput")
    o_e = nc.dram_tensor("o", [128, acols], f32, kind="ExternalOutput")

    with tile.TileContext(nc) as tc, ExitStack() as ctx:
        consts = ctx.enter_context(tc.tile_pool(name="consts", bufs=1))
        xpool = ctx.enter_context(tc.tile_pool(name="xpool", bufs=1))
        tpool = ctx.enter_context(tc.tile_pool(name="tpool", bufs=1))
        lpool = ctx.enter_context(tc.tile_pool(name="lpool", bufs=2))
        apool = ctx.enter_context(tc.tile_pool(name="apool", bufs=1))
        pp = ctx.enter_context(tc.tile_pool(name="pp", bufs=4, space="PSUM"))
        pl = ctx.enter_context(tc.tile_pool(name="pl", bufs=2, space="PSUM"))
        p2 = ctx.enter_context(tc.tile_pool(name="p2", bufs=2, space="PSUM"))

        # --- DMA triggers: x stream on the SP ring, consts on the ACT ring
        xts = {}

        def trigger_x(m):
            mg, chunk = m["mg"], m["chunk"]
            xc = xpool.tile([KP, 2, KBLK, chunk], f16, tag=f"x{mg}", name=f"x{mg}")
            off = xoffs[mg]
            nc.sync.dma_start(
                xc[:],
                xt_e[:, off : off + 2 * KBLK * chunk].rearrange(
                    "p (ck j c) -> p ck j c", ck=2, j=KBLK
                ),
            )
            xts[mg] = xc

        w0t_s = consts.tile([KP, KBLK, H], f16)
        wd_s = consts.tile([128, 2 * NPAIR, H], f16)
        woutt_s = consts.tile([128, OUT], f16)
        bias_s = consts.tile([128, L], f32)
        boutb_s = consts.tile([128, OUT * NBLK_MAX], f32)
        ident_s = consts.tile([OUT, OUT], ftp)

        trigger_x(sched[0])
        nc.scalar.dma_start(bias_s[:], bias8_e[:])
        nc.scalar.dma_start(w0t_s[:], w0t_e[:])
        trigger_x(sched[1])
        nc.scalar.dma_start(wd_s[:], wd_e[:])
        nc.scalar.dma_start(woutt_s[:], woutt_e[:])
        nc.scalar.dma_start(ident_s[:], ident_e[:] if fp16_tp else ident32_e[:])
        nc.scalar.dma_start(boutb_s[:], boutb_e[:])
        for m in sched[2 : (len(sched) if upfront_dma else W_INFLIGHT)]:
            trigger_x(m)

        # Prime ACT (loads the Silu table during the DMA prologue) and DVE.
        prim_a = consts.tile([128, 1], f32)
        prime_fn = AF.Sigmoid if silu_via_sigmoid else AF.Silu
        nc.scalar.activation(prim_a[:], bias_s[:, 0:1], prime_fn)
        prim_v = consts.tile([128, 1], f32)
        nc.vector.tensor_copy(prim_v[:], bias_s[:, 0:1])

        out_acc = apool.tile([128, acols], f32)
        ex = apool.tile([128, acols], f32)
        sm = apool.tile([128, acols // OUT], f32)
        lsm = apool.tile([128, acols // OUT], f32)
        od = apool.tile([128, acols], f32)

        # ---------------- emission-time pacing estimate -------------------
        est = dict(ns=EST_T0_NS)
        arrivals = []
        cum = 0.25e6  # consts ride a separate ring; small offset for sharing
        for m in sched:
            cum += m["mega"] * IN * 2
            arrivals.append(EST_ARR0_NS + cum / EST_BW)

        def est_add_slot(cols):
            est["ns"] += cols * EST_CYC + EST_SLOT_OVH

        # ---------------- per-megatile emission helpers -------------------
        Ts = {}

        def alloc_T(m):
            mg = m["mg"]
            Ts[mg] = [
                tpool.tile(
                    [128, 512], f16, tag=f"T{i}_{mg % NGEN}", name=f"T{i}_{mg}"
                )[:, : m["chunk"]]
                for i in range(L)
            ]

        def emit_silu(m, i, ps):
            dst = Ts[m["mg"]][i][:]
            if not silu_via_sigmoid:
                nc.scalar.activation(dst, ps[:], AF.Silu, bias=bias_s[:, i : i + 1])
            else:  # CoreSim lacks Silu; mathematically identical path
                sg = tpool.tile(
                    [128, 512], f32, tag="sg", name="sg", bufs=2
                )[:, : m["chunk"]]
                nc.scalar.activation(
                    sg[:], ps[:], AF.Sigmoid, bias=bias_s[:, i : i + 1]
                )
                nc.vector.scalar_tensor_tensor(
                    out=dst, in0=ps[:], scalar=bias_s[:, i : i + 1], in1=sg[:],
                    op0=mybir.AluOpType.add, op1=mybir.AluOpType.mult,
                )
            est["ns"] += 150.0

        def emit_l0(m):
            mg, chunk = m["mg"], m["chunk"]
            ps = pp.tile([128, 512], f32, tag="ps", name=f"ps0_{mg}")[:, :chunk]
            xc = xts[mg]
            for j in range(KBLK):
                first = j == 0
                last = j == KBLK - 1
                nc.tensor.matmul(
                    ps[0:H, :], w0t_s[:, j, :], xc[:, 0, j, :],
                    start=first, stop=last, skip_group_check=True,
                )
                nc.tensor.matmul(
                    ps[H:128, :], w0t_s[:, j, :], xc[:, 1, j, :],
                    start=first, stop=last, skip_group_check=True,
                )
                est_add_slot(chunk)
            emit_silu(m, 0, ps)

        def emit_dense(m, i):
            mg, chunk = m["mg"], m["chunk"]
            T = Ts[mg]
            pa = 64 * (i % 2)      # A-half output partitions for this layer
            pb = 64 - pa
            ps = pp.tile([128, 512], f32, tag="ps", name=f"ps{i}_{mg}")[:, :chunk]
            for m2 in range(i):
                first = m2 == 0
                last = m2 == i - 1
                p = PIDX[(i, m2)]
                nc.tensor.matmul(
                    ps[pa : pa + 64, :], wd_s[:, 2 * p, :], T[m2][:, :],
                    start=first, stop=last, skip_group_check=True,
                )
                nc.tensor.matmul(
                    ps[pb : pb + 64, :], wd_s[:, 2 * p + 1, :], T[m2][:, :],
                    start=first, stop=last, skip_group_check=True,
                )
                est_add_slot(chunk)
            emit_silu(m, i, ps)

        copy_eng = nc.vector  # gpsimd cannot access PSUM (BIR verifier)

        def emit_logits(m):
            mg, chunk = m["mg"], m["chunk"]
            T7 = Ts[mg][L - 1]  # odd layer: B in 0:64, A in 64:128
            plg = pl.tile([128, 512], f32, tag="plg", name=f"plg{mg}")[:, :chunk]
            nc.tensor.matmul(
                plg[0:OUT, :], woutt_s[H:128, :], T7[H:128, :],
                start=True, stop=True, skip_group_check=True,
            )
            nc.tensor.matmul(
                plg[64 : 64 + OUT, :], woutt_s[0:H, :], T7[0:H, :],
                start=True, stop=True, skip_group_check=True,
            )
            est_add_slot(chunk)
            lgsT = lpool.tile(
                [OUT, 1024], ftp, tag="lgsT", name=f"lg{mg}"
            )[:, : m["mega"]]
            copy_eng.tensor_copy(lgsT[:, 0:chunk], plg[0:OUT, :])
            copy_eng.tensor_copy(lgsT[:, chunk : 2 * chunk], plg[64 : 64 + OUT, :])
            return lgsT

        def emit_transpose(m, lgsT):
            mg, nblk = m["mg"], m["nblk"]
            pt = p2.tile([128, OUT * NBLK_MAX], ftp, tag="pt", name=f"pt{mg}")
            for blk in range(nblk):
                nc.tensor.matmul(
                    pt[:, blk * OUT : (blk + 1) * OUT],
                    lgsT[:, blk * 128 : (blk + 1) * 128],
                    ident_s[:],
                    is_transpose=True,
                    start=True, stop=True, skip_group_check=True,
                )
            est["ns"] += 100.0 * nblk
            nc.vector.tensor_add(
                out_acc[:, m["aoff"] : m["aoff"] + OUT * nblk],
                pt[:, : OUT * nblk],
                boutb_s[:, 0 : OUT * nblk],
            )

        def emit_softmax(c0, c1, last):
            g0, g1 = c0 // OUT, c1 // OUT
            nc.scalar.activation(ex[:, c0:c1], out_acc[:, c0:c1], AF.Exp)
            nc.vector.reduce_sum(
                out=sm[:, g0:g1],
                in_=ex[:, c0:c1].rearrange("p (g c) -> p g c", c=OUT),
                axis=mybir.AxisListType.X,
            )
            nc.scalar.activation(lsm[:, g0:g1], sm[:, g0:g1], AF.Ln)
            nc.vector.tensor_sub(
                od[:, c0:c1].rearrange("p (g c) -> p g c", c=OUT),
                out_acc[:, c0:c1].rearrange("p (g c) -> p g c", c=OUT),
                lsm[:, g0:g1].unsqueeze(2).broadcast_to([128, g1 - g0, OUT]),
            )
            nc.sync.dma_start(o_e[:, c0:c1], od[:, c0:c1])

        # ---------------- software-pipelined schedule ---------------------
        n_silu_left = nmt * L
        inflight = []
        pending = list(range(nmt))
        layer_of = {}
        softmax_split = None

        def admit():
            mg = pending.pop(0)
            m = sched[mg]
            if not upfront_dma and mg + W_INFLIGHT < len(sched):
                trigger_x(sched[mg + W_INFLIGHT])
            alloc_T(m)
            emit_l0(m)
            layer_of[mg] = 1
            inflight.append(mg)

        def can_admit():
            if not pending or len(inflight) >= W_INFLIGHT:
                return False
            if len(inflight) < 2:
                return True
            return est["ns"] >= arrivals[pending[0]] - 1200.0

        admit()
        admit()
        while inflight:
            while can_admit():
                admit()
            mg = inflight.pop(0)
            m = sched[mg]
            i = layer_of[mg]
            emit_dense(m, i)
            n_silu_left -= 1
            if n_silu_left == 0 and len(sched) > 1:
                # all silus emitted: ACT is free to swap to the Exp/Ln table;
                # everything but the final megatile's segment is softmaxed
                # under the remaining logits/transpose work
                softmax_split = sched[-1]["aoff"]
                emit_softmax(0, softmax_split, last=False)
            if i == L - 1:
                lgsT = emit_logits(m)
                emit_transpose(m, lgsT)
                if softmax_split is not None and mg == nmt - 1:
                    emit_softmax(softmax_split, acols, last=True)
            else:
                layer_of[mg] = i + 1
                inflight.append(mg)

        if softmax_split is None:
            emit_softmax(0, acols, last=True)

    _split_multi_waits(nc)
    return nc


def _split_multi_waits(nc):
    """walrus's activation encoding admits one sync-wait; hoist extras onto
    preceding same-engine NoOps (sequentially equivalent)."""
    for blk in nc.m.functions[0].blocks:
        idx = 0
        while idx < len(blk.instructions):
            inst = blk.instructions[idx]
            si = inst.sync_info
            splittable = isinstance(
                inst,
                (
                    mybir.InstActivation,
                    mybir.InstTensorCopy,
                    mybir.InstTensorTensor,
                    mybir.InstTensorReduce,
                    mybir.InstMatmult,
                    mybir.InstLdweights,
                    mybir.InstDMACopy,
                    mybir.InstMemset,
                    mybir.InstDrain,
                    mybir.InstStreamTranspose,
                ),
            )
            if splittable and si is not None and len(si.on_wait) > 1:
                extras = list(si.on_wait[:-1])
                si.on_wait = [si.on_wait[-1]]
                for w in reversed(extras):
                    nop = mybir.InstNoOp(
                        name=nc.get_next_instruction_name(), ins=[], outs=[]
                    )
                    nop.engine = inst.engine
                    nop.sync_info = mybir.SyncInfo(on_wait=[w], on_update=[])
                    nc.register_instruction(nop)
                    blk.instructions.insert(idx, nop)
                    idx += 1
            idx += 1


# ----------------------------------------------------------------------------
# Host wrapper
# ----------------------------------------------------------------------------

_CACHE = {}


def _get_nc():
    if "nc" not in _CACHE:
        _CACHE["nc"] = build_nc()
    return _CACHE["nc"]


def pack_x(x_slice, sched=None):
    """[rows, 784] fp32 -> per-core tiled layout [128, XCOLS] fp16: one slab
    per megatile, per-partition [ck, j, c] contiguous."""
    sched = SCHED if sched is None else sched
    xoffs, xcols = _xoff(sched)
    xt16 = np.zeros((KBLK, KP, x_slice.shape[0]), np.float16)
    xt16[:, :112, :] = x_slice.T.astype(np.float16).reshape(KBLK, 112, -1)
    out = np.empty((KP, xcols), np.float16)
    for m in sched:
        chunk = m["chunk"]
        off = xoffs[m["mg"]]
        blk = xt16[:, :, m["start"] : m["start"] + m["mega"]]
        # [j, p, 2*chunk] -> [p, ck, j, c]
        blk = blk.reshape(KBLK, KP, 2, chunk).transpose(1, 2, 0, 3)
        out[:, off : off + 2 * KBLK * chunk] = blk.reshape(KP, -1)
    return out


def prepare_inputs(x, W0, b0, Wh, bh, Wp, Wf, Wout, bout):
    consts = _pack_weights(W0, b0, Wh, bh, Wp, Wf, Wout, bout)
    in_maps = []
    for c in range(N_CORES):
        m = dict(consts)
        m["xt"] = pack_x(x[c * B_CORE : (c + 1) * B_CORE])
        in_maps.append(m)
    return in_maps


def _unpermute(o_core, sched=None):
    sched = SCHED if sched is None else sched
    b_core = sum(m["mega"] for m in sched)
    out = np.empty((b_core, OUT), np.float32)
    for m in sched:
        seg = o_core[:, m["aoff"] : m["aoff"] + m["nblk"] * OUT]
        seg = seg.reshape(128, m["nblk"], OUT).transpose(1, 0, 2)
        out[m["start"] : m["start"] + m["mega"]] = seg.reshape(m["mega"], OUT)
    return out


def run(inputs, trace=False, **kw):
    in_maps = prepare_inputs(**inputs)
    nc = _get_nc()
    res = run_bass_kernel_spmd(nc, in_maps, list(range(N_CORES)), trace=trace, **kw)
    out = np.empty((B, OUT), np.float32)
    for c in range(N_CORES):
        out[c * B_CORE : (c + 1) * B_CORE] = _unpermute(res.results[c]["o"])
    return out, res


def kernel(**inputs):
    out, _ = run(inputs, trace=False)
    return out
